# revision 9
# baseline (speedup 1.0000x reference)
"""Bass/Trainium2 kernel for nn_Bilinear (out[b,n,i] = enc[b,n,i,:] @ W @ hidden[b,:] + bias).

Sharding: data-parallel over B. 8 cores, one batch element each.

The kernel is DMA-bound (enc is 32 MiB/core at f32), so everything streams as
bf16 (harness gate is rel_err < 2e-2; measured bf16 error ~3e-3), halving HBM
traffic to ~18 MiB/core, and ALL math runs on the TensorE so Vector/Scalar
never pace the stream:

  host:    enc[b] is pre-transposed to enc_t [H=1024, N*I=8192] bf16 so the
           contraction dim H sits on SBUF partitions; W is fed as W.T bf16.
  stage 1: v[j] = sum_k W[j,k] h[k] on the PE: 64 matmuls with W.T chunks
           [128k, 128j] stationary and h chunks [128k, 1] moving, accumulating
           v as columns v_psum[128, 8] (v already partition-major for stage 2).
  stage 2: out[r] = sum_h enc_t[h, r] v[h]: per 128-h slab, 16 matmuls with
           v_col[:, hc] ([128, 1]) stationary and enc_t slab cols [128, 512]
           moving. The 16 row-groups accumulate into 4 PSUM banks x partitions
           {0, 32, 64, 96} (tile_position col-groups), so the whole 8192-row
           output lives in one [128, 2048] PSUM tile.
  tail:    one VectorE tensor_scalar_add applies bias while copying PSUM->SBUF;
           one 32 KiB DMA writes out[4, 2048]; host reshapes to [64, 128].

enc_t streams as 16 x 1 MiB DMAs (8 KiB/partition runs) into 8 resident SBUF
slabs (the full 16 MiB shard fits in SBUF); 1 MiB granularity keeps PE idle
gaps under the ~3.4 us HAM re-throttle window.
"""

import numpy as np
import ml_dtypes

B, N, I, H = 8, 64, 128, 1024
P = 128
NI = N * I  # 8192 rows per core
HC = H // P  # 8 h-chunks
N_CORES = 8
BF = ml_dtypes.bfloat16

_NC_CACHE = {}
LAST_RESULTS = None


def _build():
    import concourse.bacc as bacc
    import concourse.mybir as mybir
    import concourse.tile as tile

    f32 = mybir.dt.float32
    bf16 = mybir.dt.bfloat16

    nc = bacc.Bacc(
        "TRN2",
        target_bir_lowering=False,
        debug=False,
        num_devices=N_CORES,
    )
    enc_t = nc.declare_dram_parameter("enc_t", [H, NI], bf16, isOutput=False)
    wt = nc.declare_dram_parameter("wt", [H, H], bf16, isOutput=False)
    hh = nc.declare_dram_parameter("h", [P, HC], bf16, isOutput=False)
    bb = nc.declare_dram_parameter("bias", [1, 1], f32, isOutput=False)
    out = nc.declare_dram_parameter("out", [4, 4 * 512], f32, isOutput=True)

    with tile.TileContext(nc) as tc:
        with (
            tc.tile_pool(name="const", bufs=1) as const,
            tc.tile_pool(name="psum", bufs=1, space="PSUM") as psp,
        ):
            # ---- small loads ----
            h_col = const.tile([P, HC], bf16)
            nc.sync.dma_start(out=h_col[:], in_=hh[:, :])
            bias_col = const.tile([P, 1], f32)
            nc.sync.dma_start(out=bias_col[:], in_=bb[:, :].to_broadcast((P, 1)))
            warm_src = const.tile([P, 512], bf16)
            nc.vector.memset(warm_src[:], 0.0)

            # Two HWDGE rings drain in parallel (Sync=SP ring, Scalar=ACT
            # ring): W rides the ACT ring so enc streaming starts at t=0 on
            # the SP ring; enc halves are split ~evenly so slabs complete in
            # hc order and both rings finish together (9 MiB each).
            w_sb = []
            for wi in range(2):
                wtile = const.tile([P, 4, H], bf16, name=f"w{wi}")
                nc.scalar.dma_start(
                    out=wtile[:],
                    in_=wt[wi * 512 : (wi + 1) * 512, :].rearrange(
                        "(kc p) j -> p kc j", p=P
                    ),
                )
                w_sb.append(wtile)

            # ---- enc_t: 8 resident slabs, 2 x 1 MiB DMAs each ----
            e_sb = [const.tile([P, NI], bf16, name=f"e{hc}") for hc in range(HC)]
            scalar_halves = {(2, 1), (3, 1), (4, 1), (5, 1), (6, 1), (7, 0)}
            for hc in range(HC):
                for half in range(2):
                    j0 = half * (NI // 2)
                    eng = nc.scalar if (hc, half) in scalar_halves else nc.sync
                    eng.dma_start(
                        out=e_sb[hc][:, j0 : j0 + NI // 2],
                        in_=enc_t[hc * P : (hc + 1) * P, j0 : j0 + NI // 2],
                    )

            # ---- stage 1: v_psum[p, jc] = v[jc*128+p] ----
            # ~16 dummy matmuls (zeros stationary, W tile as the stream) flip
            # the PE HAM clock gate to 8/8 (needs ~3.4 us of sustained busy).
            # Reading w_sb[0] makes them start as soon as the first W tile
            # lands, so the PE is busy continuously from there through stage 1
            # and enters stage 2 at 2.4 GHz.
            warm_ps = psp.tile([P, 512], f32, name="warm")
            for _ in range(16):
                nc.tensor.matmul(
                    warm_ps[0:1, :],
                    warm_src[:, 0:1],
                    w_sb[0][:, 0, 0:512],
                    start=True,
                    stop=True,
                )

            # jc-outer: each v column's accumulation group must fully close
            # (stop=True) before the next column's start=True, because all 8
            # columns share one 2 KiB PSUM zero region and start re-marks the
            # whole region pending-zero.
            v_psum = psp.tile([P, HC], f32)
            for jc in range(HC):
                for kc in range(HC):
                    wi, kk = divmod(kc, 4)
                    lhsT = w_sb[wi][:, kk, jc * P : (jc + 1) * P]
                    nc.tensor.matmul(
                        v_psum[:, jc : jc + 1],
                        lhsT,
                        h_col[:, kc : kc + 1],
                        start=(kc == 0),
                        stop=(kc == HC - 1),
                    )
            v_col = const.tile([P, HC], bf16)
            nc.vector.tensor_copy(v_col[:], v_psum[:])

            # ---- stage 2: 16 groups of 512 rows; group g = c*4 + b lands at
            # PSUM partition 32c, bank b (columns b*512..) ----
            ps_out = psp.tile([P, 4 * 512], f32)
            for hc in range(HC):
                for half in range(2):
                    for c in (2 * half, 2 * half + 1):
                        for bk in range(4):
                            g = c * 4 + bk
                            nc.tensor.matmul(
                                ps_out[32 * c : 32 * c + 1, bk * 512 : (bk + 1) * 512],
                                v_col[:, hc : hc + 1],
                                e_sb[hc][:, g * 512 : (g + 1) * 512],
                                start=(hc == 0),
                                stop=(hc == HC - 1),
                                tile_position=(0, 32 * c),
                            )

            # ---- tail: bias add (PSUM->SBUF) + strided writeback ----
            out_sb = const.tile([P, 4 * 512], f32)
            nc.vector.tensor_scalar_add(out_sb[:], ps_out[:], bias_col[:])
            nc.sync.dma_start(out=out[:, :], in_=out_sb[0 : 3 * 32 + 1 : 32, :])
    nc.compile()
    return nc


def _get_nc():
    if "nc" not in _NC_CACHE:
        _NC_CACHE["nc"] = _build()
    return _NC_CACHE["nc"]


def kernel(hidden=None, encoder_hiddens=None, input_lengths=None, W=None, b=None):
    global LAST_RESULTS
    from concourse.bass_utils import run_bass_kernel_spmd

    hidden = np.asarray(hidden, dtype=np.float32)
    enc = np.asarray(encoder_hiddens, dtype=np.float32)
    W_ = np.asarray(W, dtype=np.float32)
    b_ = np.asarray(b, dtype=np.float32).reshape(1, 1)

    wt_bf = np.ascontiguousarray(W_.T.astype(BF))
    enc_bf = enc.astype(BF)  # [B, N, I, H]

    nc = _get_nc()
    in_maps = []
    for core in range(N_CORES):
        in_maps.append(
            {
                "enc_t": np.ascontiguousarray(enc_bf[core].reshape(NI, H).T),
                "wt": wt_bf,
                "h": np.ascontiguousarray(hidden[core].reshape(HC, P).T.astype(BF)),
                "bias": b_,
            }
        )
    res = run_bass_kernel_spmd(nc, in_maps, core_ids=list(range(N_CORES)))
    LAST_RESULTS = res
    # out[c, b*512 + r] = row (c*4+b)*512 + r of the flattened [8192] output
    out = np.stack(
        [res.results[i]["out"].reshape(NI).reshape(N, I) for i in range(N_CORES)]
    )
    return np.ascontiguousarray(out.astype(np.float32))


# revision 10
# speedup vs baseline: 1.1359x; 1.1359x over previous
"""Bass/Trainium2 kernel for nn_Bilinear (out[b,n,i] = enc[b,n,i,:] @ W @ hidden[b,:] + bias).

Sharding: data-parallel over B. 8 cores, one batch element each.

The kernel is DMA-bound (enc is 32 MiB/core at f32), so everything streams as
bf16 (harness gate is rel_err < 2e-2; measured bf16 error ~3e-3), halving HBM
traffic to ~18 MiB/core, and ALL math runs on the TensorE so Vector/Scalar
never pace the stream:

  host:    enc[b] is pre-transposed to enc_t [H=1024, N*I=8192] bf16 so the
           contraction dim H sits on SBUF partitions; W is fed as W.T bf16.
  stage 1: v[j] = sum_k W[j,k] h[k] on the PE: 64 matmuls with W.T chunks
           [128k, 128j] stationary and h chunks [128k, 1] moving, accumulating
           v as columns v_psum[128, 8] (v already partition-major for stage 2).
           jc-outer order: all 8 v columns share one 2 KiB PSUM zero region,
           so each column's accumulation group must close before the next
           start=True re-marks the region pending-zero.
  stage 2: out[r] = sum_h enc_t[h, r] v[h]: per 128-h slab, 16 matmuls with
           v_col[:, hc] ([128, 1]) stationary and enc_t slab cols [128, 512]
           moving. The 16 row-groups accumulate into 4 PSUM banks x partitions
           {0, 32, 64, 96} (tile_position col-groups), so the whole 8192-row
           output lives in one [128, 2048] PSUM tile. The bias is folded into
           the same accumulation as one rank-1 matmul (b/128 * ones) per group.
  tail:    VectorE copies PSUM banks 0-1 while ScalarE copies banks 2-3
           (parallel: different banks), two 16 KiB DMAs write out[4, 2048];
           host reshapes to [64, 128].

Schedule notes (from NTFF profiles): a single HWDGE ring sustains ~334 GB/s
(per-core HBM effective ceiling; dual-ring measured slower), so everything
rides nc.sync in issue order: h/bias, W (2 x 1 MiB), enc (8 x 2 MiB slabs).
16 dummy matmuls that read the first W tile warm the PE HAM clock gate to
8/8 during the W window so real matmuls issue at 2.4 GHz; an early dummy
activation preloads the ScalarE table set off the critical path.
"""

import numpy as np
import ml_dtypes

B, N, I, H = 8, 64, 128, 1024
P = 128
NI = N * I  # 8192 rows per core
HC = H // P  # 8 h-chunks
N_CORES = 8
BF = ml_dtypes.bfloat16

_NC_CACHE = {}
LAST_RESULTS = None


def _build():
    import concourse.bacc as bacc
    import concourse.mybir as mybir
    import concourse.tile as tile

    f32 = mybir.dt.float32
    bf16 = mybir.dt.bfloat16

    nc = bacc.Bacc(
        "TRN2",
        target_bir_lowering=False,
        debug=False,
        num_devices=N_CORES,
    )
    enc_t = nc.declare_dram_parameter("enc_t", [H, NI], bf16, isOutput=False)
    wt = nc.declare_dram_parameter("wt", [H, H], bf16, isOutput=False)
    hh = nc.declare_dram_parameter("h", [P, HC], bf16, isOutput=False)
    bb = nc.declare_dram_parameter("bias", [1, 1], f32, isOutput=False)
    out = nc.declare_dram_parameter("out", [4, 4 * 512], f32, isOutput=True)

    with tile.TileContext(nc) as tc:
        with (
            tc.tile_pool(name="const", bufs=1) as const,
            tc.tile_pool(name="psum", bufs=1, space="PSUM") as psp,
        ):
            # ---- small loads + local constants ----
            h_col = const.tile([P, HC], bf16)
            nc.sync.dma_start(out=h_col[:], in_=hh[:, :])
            bias_col = const.tile([P, 1], f32)
            nc.sync.dma_start(out=bias_col[:], in_=bb[:, :].to_broadcast((P, 1)))
            ones_sb = const.tile([P, 512], bf16)
            nc.vector.memset(ones_sb[:], 1.0)
            # bias/128 per partition; summed back to b by a rank-1 matmul
            bias_bf = const.tile([P, 1], bf16)
            nc.vector.tensor_scalar_mul(bias_bf[:], bias_col[:], 1.0 / P)
            # preload the ScalarE activation table set (~2.7 us) off the
            # critical path so the tail Copy doesn't pay it
            act_warm = const.tile([P, 1], f32)
            nc.scalar.activation(
                act_warm[:], bias_col[:], mybir.ActivationFunctionType.Copy
            )

            # ---- W.T as two 1 MiB DMAs, kc-major in the free dim ----
            w_sb = []
            for wi in range(2):
                wtile = const.tile([P, 4, H], bf16, name=f"w{wi}")
                nc.sync.dma_start(
                    out=wtile[:],
                    in_=wt[wi * 512 : (wi + 1) * 512, :].rearrange(
                        "(kc p) j -> p kc j", p=P
                    ),
                )
                w_sb.append(wtile)

            # ---- enc_t: 8 resident slabs, one 2 MiB DMA each ----
            e_sb = [const.tile([P, NI], bf16, name=f"e{hc}") for hc in range(HC)]
            for hc in range(HC):
                nc.sync.dma_start(
                    out=e_sb[hc][:],
                    in_=enc_t[hc * P : (hc + 1) * P, :],
                )

            # ---- PE warm-up: starts when the first W tile lands ----
            warm_ps = psp.tile([P, 512], f32, name="warm")
            for _ in range(16):
                nc.tensor.matmul(
                    warm_ps[0:1, :],
                    ones_sb[:, 0:1],
                    w_sb[0][:, 0, 0:512],
                    start=True,
                    stop=True,
                )

            # ---- stage 1: v_psum[p, jc] = v[jc*128+p] ----
            v_psum = psp.tile([P, HC], f32)
            for jc in range(HC):
                for kc in range(HC):
                    wi, kk = divmod(kc, 4)
                    lhsT = w_sb[wi][:, kk, jc * P : (jc + 1) * P]
                    nc.tensor.matmul(
                        v_psum[:, jc : jc + 1],
                        lhsT,
                        h_col[:, kc : kc + 1],
                        start=(kc == 0),
                        stop=(kc == HC - 1),
                    )
            v_col = const.tile([P, HC], bf16)
            nc.vector.tensor_copy(v_col[:], v_psum[:])

            # ---- stage 2: group g = c*4 + bk -> PSUM partition 32c, bank bk.
            # Slab 7 runs bank-major so banks drain to SBUF in order. ----
            ps_out = psp.tile([P, 4 * 512], f32)

            def mm(c, bk, hc):
                g = c * 4 + bk
                nc.tensor.matmul(
                    ps_out[32 * c : 32 * c + 1, bk * 512 : (bk + 1) * 512],
                    v_col[:, hc : hc + 1],
                    e_sb[hc][:, g * 512 : (g + 1) * 512],
                    start=(hc == 0),
                    stop=(hc == HC - 1),
                    tile_position=(0, 32 * c),
                )

            for hc in range(HC - 1):
                for c in range(4):
                    for bk in range(4):
                        mm(c, bk, hc)
                if hc == 3:
                    # fold the bias into each group's accumulation:
                    # out[g-row, :] += sum_p (b/128) * 1
                    for c in range(4):
                        for bk in range(4):
                            nc.tensor.matmul(
                                ps_out[
                                    32 * c : 32 * c + 1, bk * 512 : (bk + 1) * 512
                                ],
                                bias_bf[:],
                                ones_sb[:],
                                start=False,
                                stop=False,
                                tile_position=(0, 32 * c),
                            )
            for bk in range(4):
                for c in range(4):
                    mm(c, bk, HC - 1)

            # ---- tail: parallel PSUM->SBUF drain (different banks), 2 DMAs ----
            out_sb = const.tile([P, 4 * 512], f32)
            nc.vector.tensor_copy(out_sb[:, 0:1024], ps_out[:, 0:1024])
            nc.scalar.activation(
                out_sb[:, 1024:2048],
                ps_out[:, 1024:2048],
                mybir.ActivationFunctionType.Copy,
            )
            nc.sync.dma_start(
                out=out[:, 0:1024], in_=out_sb[0 : 3 * 32 + 1 : 32, 0:1024]
            )
            nc.sync.dma_start(
                out=out[:, 1024:2048], in_=out_sb[0 : 3 * 32 + 1 : 32, 1024:2048]
            )
    nc.compile()
    return nc


def _get_nc():
    if "nc" not in _NC_CACHE:
        _NC_CACHE["nc"] = _build()
    return _NC_CACHE["nc"]


def kernel(hidden=None, encoder_hiddens=None, input_lengths=None, W=None, b=None):
    global LAST_RESULTS
    from concourse.bass_utils import run_bass_kernel_spmd

    hidden = np.asarray(hidden, dtype=np.float32)
    enc = np.asarray(encoder_hiddens, dtype=np.float32)
    W_ = np.asarray(W, dtype=np.float32)
    b_ = np.asarray(b, dtype=np.float32).reshape(1, 1)

    wt_bf = np.ascontiguousarray(W_.T.astype(BF))
    enc_bf = enc.astype(BF)  # [B, N, I, H]

    nc = _get_nc()
    in_maps = []
    for core in range(N_CORES):
        in_maps.append(
            {
                "enc_t": np.ascontiguousarray(enc_bf[core].reshape(NI, H).T),
                "wt": wt_bf,
                "h": np.ascontiguousarray(hidden[core].reshape(HC, P).T.astype(BF)),
                "bias": b_,
            }
        )
    res = run_bass_kernel_spmd(nc, in_maps, core_ids=list(range(N_CORES)))
    LAST_RESULTS = res
    # out[c, b*512 + r] = row (c*4+b)*512 + r of the flattened [8192] output
    out = np.stack(
        [res.results[i]["out"].reshape(NI).reshape(N, I) for i in range(N_CORES)]
    )
    return np.ascontiguousarray(out.astype(np.float32))
